# revision 1
# baseline (speedup 1.0000x reference)
# Trainium2 Bass kernel for nn_DenoisingLossDDP (NT-Xent + shifted MSE).
#
# Reference math: K=N*BS=2048 rows of h (D=4096); sn = row/||row||;
# sim2 = 2*(sn@sn.T); per row i: negsum_i = sum_j e^{sim2_ij} minus the 16
# per-128-block diagonal entries; loss_h = sum over 15 positives of
# [ln(negsum + e^pos) - pos] / (K*15); loss_pairs = mean((pic - dec_shift)^2).
#
# Design:
#  * All inputs quantized to fp8e4 on the host (rel err ~1.5e-4, tol 2e-2).
#  * Host pre-transposes h to hT [D, K] and ROTATES each core's columns so
#    its own 256 columns sit at position 0: lhsT is then a static slice of
#    the streamed rhs tile (SPMD-safe) and the self-block of the diag is
#    always block m, so the selfmask is static.  Layout [16, 128, 2, 2048]
#    gives one 4KB-contiguous DMA descriptor per partition per k-pair tile.
#  * Gram on raw quantized h, fp8 DoubleRow matmuls (256 contraction rows
#    per instruction), full [2m x 4chunk] slice resident in all 8 PSUM
#    banks across the k-stream.
#  * Normalization post-matmul: sim2 = (G * invj_bcast) * (2*inv_i), row
#    factor folded into the Exp's per-partition scale.  Norms: own 256 rows
#    via ACT square+accum on a small row-major slice, 8-core AllGather of
#    inv (1KB) + broadcast-DMA back.  The collective's ~50us latency
#    overlaps the matmul stream.
#  * MSE: fp8 pic pairs, one 4KB-contiguous DMA per chunk, subtract split
#    DVE (early chunks) / GpSimd (late), square+accum on ACT.

import numpy as np
from contextlib import ExitStack

import ml_dtypes
from concourse import bacc, bass, tile, mybir
from concourse import bass_utils

N, BS, D = 16, 128, 4096
K = N * BS                      # 2048
C3 = 3 * 64 * 64                # 12288
NCORES = 8
RPC = K // NCORES               # 256 rows per core
NPC = N // NCORES               # 2 pic slices per core
NDT = D // 256                  # 16 double-k-tiles
MSE_DEN = float(N * BS * C3)
NT_DEN = float(K * (N - 1))
PIC_CHUNK = 2048
NPICS = 2 * C3 // PIC_CHUNK     # 12 chunks
OUT_COLS = 16                   # 0..11 mse partials, 12..13 nt partials

F32 = mybir.dt.float32
BF16 = mybir.dt.bfloat16
FP8 = mybir.dt.float8e4
NP_FP8 = ml_dtypes.float8_e4m3
AF = mybir.ActivationFunctionType
OP = mybir.AluOpType

DOUBLE_ROW = True               # fp8 DoubleRow matmul


def _body(tc, out, htr, hslr, pr):
    nc = tc.nc
    with ExitStack() as ctx:
        small = ctx.enter_context(tc.tile_pool(name="small", bufs=1))
        htp = ctx.enter_context(tc.tile_pool(name="htp", bufs=4))
        picsp = ctx.enter_context(tc.tile_pool(name="pics", bufs=3))
        diffp = ctx.enter_context(tc.tile_pool(name="diff", bufs=3))
        simp = ctx.enter_context(tc.tile_pool(name="simp", bufs=2))
        maskp = ctx.enter_context(tc.tile_pool(name="maskp", bufs=2))
        psump = ctx.enter_context(
            tc.tile_pool(name="psum", bufs=1, space=bass.MemorySpace.PSUM)
        )
        dramp = ctx.enter_context(tc.tile_pool(name="dram", bufs=1, space="DRAM"))

        # ---- persistent tiles ----
        hsb = small.tile([128, 2, D], FP8, name="hsb", tag="hsb")
        invb = small.tile([128, K], F32, name="invb", tag="invb")
        junkA = small.tile([128, D], BF16, name="junkA", tag="junkA")
        ejunk = small.tile([128, K], BF16, name="ejunk", tag="ejunk")
        dmask = small.tile([128, N, 128], F32, name="dmask", tag="dmask")
        smb = small.tile([128, 2, N], F32, name="smb", tag="smb")
        norms2 = small.tile([128, 2], F32, name="norms2", tag="norms2")
        inv = small.tile([128, 2], F32, name="inv", tag="inv")
        inv2 = small.tile([128, 2], F32, name="inv2", tag="inv2")
        rowsum = small.tile([128, 8], F32, name="rowsum", tag="rowsum")
        posw = small.tile([128, 2, N], F32, name="posw", tag="posw")
        pos = small.tile([128, 2, N], F32, name="pos", tag="pos")
        eP = small.tile([128, 2, N], F32, name="eP", tag="eP")
        tmp16 = small.tile([128, 2, N], F32, name="tmp16", tag="tmp16")
        tot = small.tile([128, 2], F32, name="tot", tag="tot")
        dsum = small.tile([128, 2], F32, name="dsum", tag="dsum")
        negsum = small.tile([128, 2], F32, name="negsum", tag="negsum")
        acc = small.tile([128, OUT_COLS], F32, name="acc", tag="acc")

        psm = {
            m: psump.tile([128, 4, 512], F32, name=f"psm{m}", tag=f"psm{m}")
            for m in range(2)
        }

        inv_in = dramp.tile([2, 128], F32, name="inv_in", tag="inv_in")
        inv_all = dramp.tile([NCORES, 2, 128], F32, name="inv_all", tag="inv_all")

        # ---- setup ----
        nc.gpsimd.memset(acc[:, :], 0.0)
        nc.gpsimd.memset(dmask[:, :, :], 0.0)
        nc.gpsimd.affine_select(
            out=dmask[:, :, :],
            in_=dmask[:, :, :],
            compare_op=OP.not_equal,
            fill=1.0,
            base=0,
            pattern=[[0, N], [-1, 128]],
            channel_multiplier=1,
        )
        # static selfmask: after rotation the self block of m-tile m is m
        nc.gpsimd.memset(smb[:, :, :], 1.0)
        nc.gpsimd.memset(smb[:, 0, 0:1], 0.0)
        nc.gpsimd.memset(smb[:, 1, 1:2], 0.0)

        # ---- own-row norms + 8-core AllGather of inv ----
        nc.sync.dma_start(out=hsb[:, :, :], in_=hslr[:, :, :])
        for u in range(2):
            nc.scalar.activation(
                out=junkA[:, :], in_=hsb[:, u, :], func=AF.Square,
                accum_out=norms2[:, u : u + 1],
            )
        nc.vector.reciprocal(inv[:, :], norms2[:, :])
        nc.scalar.activation(out=inv[:, :], in_=inv[:, :], func=AF.Sqrt)
        nc.vector.tensor_scalar(
            out=inv2[:, :], in0=inv[:, :], scalar1=2.0, scalar2=None, op0=OP.mult
        )
        # inv gather chain entirely on gpsimd (sync stays on the ht stream)
        for u in range(2):
            nc.gpsimd.dma_start(out=inv_in[u, :], in_=inv[:, u : u + 1])
        nc.gpsimd.collective_compute(
            "AllGather",
            OP.bypass,
            replica_groups=[list(range(NCORES))],
            ins=[inv_in.opt()],
            outs=[inv_all.opt()],
        )
        invb_src = bass.AP(
            tensor=inv_all.tensor, offset=inv_all.offset, ap=[[0, 128], [1, K]]
        )
        nc.gpsimd.dma_start(out=invb[:, :], in_=invb_src)

        # ---- MSE pic chunk ----
        def do_picchunk(idx):
            pt = picsp.tile([128, 2, PIC_CHUNK], FP8, name="pt", tag="pt")
            teng = nc.scalar if idx % 2 == 0 else nc.sync
            teng.dma_start(out=pt[:, :, :], in_=pr[idx])
            df = diffp.tile([128, PIC_CHUNK], BF16, name="df", tag="df")
            # all subs on DVE (idle in the stream window); Pool subs at
            # 3.8us each were landing at 88-104us and polluting the tail
            nc.vector.tensor_tensor(
                out=df[:, :], in0=pt[:, 0, :], in1=pt[:, 1, :], op=OP.subtract
            )
            nc.scalar.activation(
                out=junkA[:, :PIC_CHUNK], in_=df[:, :], func=AF.Square,
                accum_out=acc[:, idx : idx + 1],
            )

        # ---- streamed Gram: 16 double-k-tiles through 8 PSUM banks ----
        pi = 0
        for t in range(NDT):
            dt_ = htp.tile([128, 2, K], FP8, name="dt", tag="dt")
            nc.sync.dma_start(out=dt_[:, :, :], in_=htr[t])
            for m in range(2):
                for c in range(4):
                    if DOUBLE_ROW:
                        nc.tensor.matmul(
                            psm[m][:, c, :],
                            lhsT=dt_[:, :, 128 * m : 128 * (m + 1)],
                            rhs=dt_[:, :, 512 * c : 512 * (c + 1)],
                            start=(t == 0),
                            stop=(t == NDT - 1),
                            perf_mode=mybir.MatmulPerfMode.DoubleRow,
                        )
                    else:
                        for i in range(2):
                            nc.tensor.matmul(
                                psm[m][:, c, :],
                                lhsT=dt_[:, i, 128 * m : 128 * (m + 1)],
                                rhs=dt_[:, i, 512 * c : 512 * (c + 1)],
                                start=(t == 0 and i == 0),
                                stop=(t == NDT - 1 and i == 1),
                            )
            # compress the pic schedule to finish by dtile 11 so all MSE
            # processing completes before the collective does
            while pi < min(NPICS, (t + 1) * NPICS // 12):
                do_picchunk(pi)
                pi += 1

        # ---- post: scale, exp-rowsum, diag extraction (one pass per m) ----
        for m in range(2):
            simw = simp.tile([128, K], F32, name="simw", tag="simw")
            nc.vector.tensor_tensor(
                out=simw[:, :],
                in0=psm[m][:, :, :].rearrange("p c x -> p (c x)"),
                in1=invb[:, :],
                op=OP.mult,
            )
            nc.scalar.activation(
                out=ejunk[:, :], in_=simw[:, :], func=AF.Exp,
                scale=inv2[:, m : m + 1],
                accum_out=tot[:, m : m + 1],
            )
            masked = maskp.tile([128, N, 128], F32, name="masked", tag="masked")
            nc.gpsimd.tensor_tensor(
                out=masked[:, :, :],
                in0=simw[:, :].rearrange("p (b x) -> p b x", x=128),
                in1=dmask[:, :, :],
                op=OP.mult,
            )
            nc.vector.tensor_reduce(
                out=posw[:, m, :],
                in_=masked[:, :, :],
                axis=mybir.AxisListType.X,
                op=OP.add,
            )

        # pos = 2*sim diag entries (posw has only the invj factor so far)
        for m in range(2):
            nc.vector.tensor_scalar(
                out=pos[:, m, :], in0=posw[:, m, :],
                scalar1=inv2[:, m : m + 1], scalar2=None, op0=OP.mult,
            )
        nc.scalar.activation(out=eP[:, :, :], in_=pos[:, :, :], func=AF.Exp)
        nc.vector.tensor_reduce(
            out=dsum[:, :], in_=eP[:, :, :], axis=mybir.AxisListType.X, op=OP.add
        )
        nc.vector.tensor_tensor(
            out=negsum[:, :], in0=tot[:, :], in1=dsum[:, :], op=OP.subtract
        )
        for m in range(2):
            nc.vector.tensor_scalar(
                out=tmp16[:, m, :], in0=eP[:, m, :],
                scalar1=negsum[:, m : m + 1], scalar2=None, op0=OP.add,
            )
        nc.scalar.activation(out=tmp16[:, :, :], in_=tmp16[:, :, :], func=AF.Ln)
        nc.vector.tensor_tensor(
            out=tmp16[:, :, :], in0=tmp16[:, :, :], in1=pos[:, :, :], op=OP.subtract
        )
        nc.vector.tensor_tensor(
            out=tmp16[:, :, :], in0=tmp16[:, :, :], in1=smb[:, :, :], op=OP.mult
        )
        nc.vector.tensor_reduce(
            out=acc[:, 12:14],
            in_=tmp16[:, :, :],
            axis=mybir.AxisListType.X,
            op=OP.add,
        )

        nc.sync.dma_start(out=out[:, :], in_=acc[:, :])


_CACHE = {}


def _build():
    if "nc" in _CACHE:
        return _CACHE["nc"]
    nc = bacc.Bacc("TRN2", target_bir_lowering=False, debug=False, num_devices=NCORES)
    htr = nc.dram_tensor("htr", [NDT, 128, 2, K], FP8, kind="ExternalInput").ap()
    hslr = nc.dram_tensor("hslr", [128, 2, D], FP8, kind="ExternalInput").ap()
    pr = nc.dram_tensor("pr", [NPICS, 128, 2, PIC_CHUNK], FP8, kind="ExternalInput").ap()
    out = nc.dram_tensor("out", [128, OUT_COLS], F32, kind="ExternalOutput").ap()
    with tile.TileContext(nc) as tc:
        _body(tc, out, htr, hslr, pr)
    nc.compile()
    _CACHE["nc"] = nc
    return nc


def make_in_maps(pic_set, dec_pics, h):
    hf = np.ascontiguousarray(h.reshape(K, D), dtype=np.float32)
    ht8 = np.ascontiguousarray(hf.T).astype(NP_FP8)          # [D, K]
    pic = pic_set.reshape(N, BS, C3)
    dec = dec_pics.reshape(N, BS, C3)
    in_maps = []
    for c in range(NCORES):
        # rotate columns so own 256 cols sit at position 0, pack for
        # 4KB-contiguous per-partition DMA lines: [t, p, i, j]
        rot = np.roll(ht8, -RPC * c, axis=1)
        htr = np.ascontiguousarray(
            rot.reshape(NDT, 2, 128, K).transpose(0, 2, 1, 3)
        )
        # own rows, packed [p, u, d]
        hsl = hf[RPC * c : RPC * (c + 1)].astype(NP_FP8)
        hslr = np.ascontiguousarray(
            hsl.reshape(2, 128, D).transpose(1, 0, 2)
        )
        ns = [NPC * c + i for i in range(NPC)]
        picp = pic[ns].reshape(NPC * BS, C3)
        picd = dec[[(n + 1) % N for n in ns]].reshape(NPC * BS, C3)
        # chunks [12, 128, 2, 2048]: chunk idx = rt*6+ch over rows 128rt+p
        ppair = np.stack([picp, picd], axis=1).astype(NP_FP8)  # [256, 2, C3]
        prr = np.ascontiguousarray(
            ppair.reshape(2, 128, 2, NPICS // 2, PIC_CHUNK)
            .transpose(0, 3, 1, 2, 4)
            .reshape(NPICS, 128, 2, PIC_CHUNK)
        )
        in_maps.append({"htr": htr, "hslr": hslr, "pr": prr})
    return in_maps


def combine(results):
    a = np.stack([r["out"] for r in results])  # (8, 128, 16)
    mse = a[:, :, :NPICS].sum(dtype=np.float64) / MSE_DEN
    nt = a[:, :, 12:14].sum(dtype=np.float64) / NT_DEN
    return np.float32(mse + nt)


def run(pic_set, dec_pics, h, trace=False):
    nc = _build()
    in_maps = make_in_maps(pic_set, dec_pics, h)
    res = bass_utils.run_bass_kernel_spmd(
        nc, in_maps, core_ids=list(range(NCORES)), trace=trace
    )
    return combine(res.results), res


def kernel(pic_set, dec_pics, h):
    val, _ = run(pic_set, dec_pics, h, trace=False)
    return np.array(val, dtype=np.float32)

